# revision 6
# baseline (speedup 1.0000x reference)
"""Trainium2 Bass kernel for nn_Encoder_55490977464569 (binary-tree GRU encoder).

Strategy (v5)
-------------
Data-parallel over batch: B=16 -> 2 batch columns per NeuronCore, zero
collectives. Each core runs its whole tree (32767 nodes) leaves->root with all
hidden states resident in SBUF (bf16); only `targets` is streamed in.

Layout: feature-major [128 features (partitions), node*batch columns], each
level's nodes stored in BIT-REVERSED in-level order. With bit-reversal at
every level, the children of parent tile [t0, t0+T) are planeL =
child[:, t0:t0+T] and planeR = child[:, R_parent+t0 : +T] (both contiguous),
and the parent's h is written back contiguously.

The ScalarE ACTIVATE stream is the bottleneck: every PSUM column needs
exactly one ACT (PSUM->SBUF drain + nonlinearity fused), at 1 col/cycle
@1.2GHz + ~180cy/instr overhead. v5 minimizes instruction count with wide
merged ACTs on 4-bank PSUM tiles:
  - leaves first (not interleaved), one [128,2048] PSUM tile [zA|zB|nA|nB]
    per unit (alternating between the two 4-bank tiles for 2-deep
    pipelining): ONE sigmoid ACT [1024] + ONE tanh ACT [1024], then
    h = zz*n as a single two-region TT.
  - interior pair (2 tiles x T=512 parents): psr [128,2048] holds
    [rlA|rrA|rlB|rrB] -> ONE sigmoid ACT [2048]; psZN [128,2048] holds
    [zA|zB|nA|nB] -> ONE sigmoid [1024] + ONE tanh [1024].
  - W_hz/w_iz are negated on the host so zz = sigmoid(+pre) at scale=1.
Blend h = cs + zz*(n - cs) runs as 3 plain [1024]-wide TTs (DVE 2x mode).
"""

import sys

if "/opt/trn_rl_repo" not in sys.path:
    sys.path.insert(0, "/opt/trn_rl_repo")
if "/opt/trn_rl_repo/concourse" not in sys.path:
    sys.path.insert(0, "/opt/trn_rl_repo/concourse")

import numpy as np
import ml_dtypes

from concourse import bass, mybir, tile, bacc
from concourse import bass_utils

BF16NP = ml_dtypes.bfloat16
F32 = mybir.dt.float32
BF16 = mybir.dt.bfloat16

N_CORES = 8
DEPTH = 15
HID = 128
IN_DIM = 32
OUT_DIM = 64
BATCH = 16
B_LOCAL = BATCH // N_CORES

T_TILE = 512
SMALL_MAX_LVL = 7

ADD = mybir.AluOpType.add
SUB = mybir.AluOpType.subtract
MULT = mybir.AluOpType.mult
TANH = mybir.ActivationFunctionType.Tanh
SIGM = mybir.ActivationFunctionType.Sigmoid


def _R(l):
    return 2**l * B_LOCAL


def _bitrev(n_bits):
    n = 1 << n_bits
    p = np.zeros(n, dtype=np.int64)
    for i in range(n):
        r = 0
        x = i
        for _ in range(n_bits):
            r = (r << 1) | (x & 1)
            x >>= 1
        p[i] = r
    return p


def build_program(with_mask=False, with_bias=False):
    nc = bacc.Bacc("TRN2", target_bir_lowering=False, debug=False,
                   num_devices=1)
    leaf = DEPTH - 1

    int_lvls = list(range(DEPTH - 2, SMALL_MAX_LVL, -1))
    int_off = {}
    off = 0
    for l in int_lvls:
        int_off[l] = off
        off += _R(l)
    xint_d = nc.dram_tensor("xint", [128, off], BF16, kind="ExternalInput")
    n_units = _R(leaf) // (2 * T_TILE)
    xleaf_d = nc.dram_tensor("xleaf", [128, n_units * T_TILE], BF16,
                             kind="ExternalInput")
    small_cols = sum(_R(l) for l in range(SMALL_MAX_LVL + 1))
    xsmall_d = nc.dram_tensor("xsmall", [128, small_cols], BF16,
                              kind="ExternalInput")
    wcat_d = nc.dram_tensor("wcat", [128, 5 * HID], BF16, kind="ExternalInput")
    w_out_d = nc.dram_tensor("w_out", [HID, 2 * OUT_DIM], F32,
                             kind="ExternalInput")
    out_d = nc.dram_tensor("out", [HID, B_LOCAL], F32, kind="ExternalOutput")
    if with_bias:
        bias_d = nc.dram_tensor("biases", [HID, 6], F32, kind="ExternalInput")
    if with_mask:
        total_z = sum(_R(l) for l in range(DEPTH))
        mask_d = nc.dram_tensor("mask_bc", [HID, total_z], BF16,
                                kind="ExternalInput")
        mask_off = {}
        moff = 0
        for l in range(DEPTH):
            mask_off[l] = moff
            moff += _R(l)

    from contextlib import ExitStack
    with tile.TileContext(nc) as tc, ExitStack() as stack:
        consts = stack.enter_context(tc.tile_pool(name="consts", bufs=1))
        hpool = stack.enter_context(tc.tile_pool(name="hpool", bufs=1))
        xpool = stack.enter_context(tc.tile_pool(name="xpool", bufs=6))
        apool = stack.enter_context(tc.tile_pool(name="apool", bufs=4))
        tpool = stack.enter_context(tc.tile_pool(name="tpool", bufs=4))
        pspool = stack.enter_context(tc.tile_pool(name="pspool", bufs=1,
                                                  space="PSUM"))
        opool = stack.enter_context(tc.tile_pool(name="opool", bufs=1))

        wcat_sb = consts.tile([128, 5 * HID], BF16, name="wcat_sb",
                              tag="wcat_sb")
        nc.sync.dma_start(out=wcat_sb, in_=wcat_d.ap())
        w_hr = wcat_sb[:, 0 * HID:1 * HID]
        w_hz = wcat_sb[:, 1 * HID:2 * HID]   # negated on host
        w_hn = wcat_sb[:, 2 * HID:3 * HID]
        wx = wcat_sb[:, 3 * HID:4 * HID]   # [w_ir; w_ir; -w_iz; w_in]
        wl = wcat_sb[:, 4 * HID:5 * HID]   # [-w_iz; -w_iz; w_in; w_in]
        w_out = consts.tile([HID, 2 * OUT_DIM], F32, name="w_out_sb",
                            tag="w_out_sb")
        xsmall = consts.tile([128, small_cols], BF16, name="xsmall",
                             tag="xsmall")
        # (their DMAs are emitted after the first leaf units, off the
        # startup critical path)
        small_off = {}
        soff = 0
        for l in range(SMALL_MAX_LVL, -1, -1):
            small_off[l] = soff
            soff += _R(l)
        if with_bias:
            bias_sb = consts.tile([HID, 6], F32, name="bias_sb", tag="bias_sb")
            nc.sync.dma_start(out=bias_sb, in_=bias_d.ap())
            b_r = bias_sb[:, 0:1]      # b_ir + b_hr
            b_zneg = bias_sb[:, 1:2]   # -(b_iz + b_hz)
            b_n = bias_sb[:, 2:3]      # b_in + b_hn
            b_out = bias_sb[:, 3:4]
            b_lz = bias_sb[:, 4:5]     # -b_iz  (leaf z)
            b_ln = bias_sb[:, 5:6]     # b_in   (leaf n)

        h_lvl = [hpool.tile([HID, _R(l)], BF16, name=f"h_{l}", tag=f"h_{l}")
                 for l in range(DEPTH)]

        def mask_mul(view, lvl, col0, width):
            m_sb = tpool.tile([HID, width], BF16, name="m_sb", tag="m_sb")
            nc.sync.dma_start(
                out=m_sb,
                in_=mask_d.ap()[:, mask_off[lvl] + col0:
                                mask_off[lvl] + col0 + width])
            nc.vector.tensor_mul(view, view, m_sb)

        def kw_b(b):
            return dict(bias=b) if with_bias else {}

        def ps_tile(i):
            if i % 2 == 0:
                return pspool.tile([HID, 2048], F32, name="psA", tag="psr")
            return pspool.tile([HID, 2048], F32, name="psB", tag="psz")

        # ---------------- leaf units ----------------
        # unit k covers tiles A = cols [kT,(k+1)T) and B = half + same.
        def leaf_unit(k):
            T = T_TILE
            xp = xpool.tile([128, T], BF16, name="xp_leaf", tag="xp")
            nc.sync.dma_start(out=xp, in_=xleaf_d.ap()[:, k * T:(k + 1) * T])
            ps = ps_tile(k)
            # [zA | zB | nA | nB]; strips of xp are [xA; xB; xA; xB]
            for s in range(4):
                nc.tensor.matmul(ps[:, s * T:(s + 1) * T],
                                 wl[32 * s:32 * (s + 1)],
                                 xp[32 * s:32 * (s + 1)],
                                 start=True, stop=True,
                                 tile_position=(32 * s, 0))
            zn = apool.tile([HID, 4 * T], BF16, name="zn_leaf", tag="act")
            nc.scalar.activation(zn[:, 0:2 * T], ps[:, 0:2 * T], SIGM,
                                 **kw_b(b_lz if with_bias else None))
            nc.scalar.activation(zn[:, 2 * T:4 * T], ps[:, 2 * T:4 * T], TANH,
                                 **kw_b(b_ln if with_bias else None))
            hv = h_lvl[leaf]
            half = _R(leaf) // 2
            hv2 = hv.rearrange("p (g f) -> p g f", g=2)[:, :, k * T:(k + 1) * T]
            zzv = zn[:, 0:2 * T].rearrange("p (g f) -> p g f", g=2)
            nv = zn[:, 2 * T:4 * T].rearrange("p (g f) -> p g f", g=2)
            nc.vector.tensor_mul(hv2, zzv, nv)
            if with_mask:
                mask_mul(hv[:, k * T:(k + 1) * T], leaf, k * T, T)
                mask_mul(hv[:, half + k * T:half + (k + 1) * T], leaf,
                         half + k * T, T)

        # ---------------- interior pair (2 tiles of T parents) ----------
        def pair_unit(l, j):
            T = T_TILE
            t0 = 2 * j * T
            ch = h_lvl[l + 1]
            R = _R(l)
            hl2 = ch[:, t0:t0 + 2 * T]
            hr2 = ch[:, R + t0:R + t0 + 2 * T]
            if l > SMALL_MAX_LVL:
                xpA = xpool.tile([128, T], BF16, name="xp_intA", tag="xp")
                nc.sync.dma_start(
                    out=xpA, in_=xint_d.ap()[:, int_off[l] + t0:
                                             int_off[l] + t0 + T])
                xpB = xpool.tile([128, T], BF16, name="xp_intB", tag="xp")
                nc.sync.dma_start(
                    out=xpB, in_=xint_d.ap()[:, int_off[l] + t0 + T:
                                             int_off[l] + t0 + 2 * T])
            else:
                xpA = xsmall[:, small_off[l] + t0:small_off[l] + t0 + T]
                xpB = xsmall[:, small_off[l] + t0 + T:small_off[l] + t0 + 2 * T]
            cs = tpool.tile([HID, 2 * T], BF16, name="cs", tag="cs")
            nc.vector.tensor_add(cs, hl2, hr2)
            # r phase: psr = [rlA | rrA | rlB | rrB]
            psr = pspool.tile([HID, 2048], F32, name="psr", tag="psr")
            for i, xp in enumerate((xpA, xpB)):
                o = 2 * i * T
                for s in range(2):
                    nc.tensor.matmul(psr[:, o + s * T:o + (s + 1) * T],
                                     wx[32 * s:32 * (s + 1)],
                                     xp[32 * s:32 * (s + 1)],
                                     start=True, stop=False,
                                     tile_position=(32 * s, 0))
                nc.tensor.matmul(psr[:, o:o + T], w_hr,
                                 ch[:, t0 + i * T:t0 + (i + 1) * T],
                                 start=False, stop=True)
                nc.tensor.matmul(psr[:, o + T:o + 2 * T], w_hr,
                                 ch[:, R + t0 + i * T:R + t0 + (i + 1) * T],
                                 start=False, stop=True)
            r_sb = apool.tile([HID, 4 * T], BF16, name="r_sb", tag="act")
            nc.scalar.activation(r_sb, psr, SIGM,
                                 **kw_b(b_r if with_bias else None))
            # t2 = r * h_child (two-region child APs, one TT per tile)
            t2 = tpool.tile([HID, 4 * T], BF16, name="t2", tag="t2")
            ch2 = ch.rearrange("p (g f) -> p g f", g=2)
            for i in range(2):
                sl = slice(2 * i * T, (2 * i + 2) * T)
                nc.vector.tensor_mul(
                    t2[:, sl].rearrange("p (g f) -> p g f", g=2),
                    r_sb[:, sl].rearrange("p (g f) -> p g f", g=2),
                    ch2[:, :, t0 + i * T:t0 + (i + 1) * T])
            # zn phase: psZN = [zA | zB | nA | nB]
            psZN = pspool.tile([HID, 2048], F32, name="psZN", tag="psz")
            for i, xp in enumerate((xpA, xpB)):
                nc.tensor.matmul(psZN[:, i * T:(i + 1) * T],
                                 wx[64:96], xp[64:96],
                                 start=True, stop=False,
                                 tile_position=(64, 0))
                nc.tensor.matmul(psZN[:, (2 + i) * T:(3 + i) * T],
                                 wx[96:128], xp[96:128],
                                 start=True, stop=False,
                                 tile_position=(96, 0))
            for i in range(2):
                nc.tensor.matmul(psZN[:, i * T:(i + 1) * T], w_hz,
                                 cs[:, i * T:(i + 1) * T],
                                 start=False, stop=True)
                nc.tensor.matmul(psZN[:, (2 + i) * T:(3 + i) * T], w_hn,
                                 t2[:, 2 * i * T:(2 * i + 1) * T],
                                 start=False, stop=False)
                nc.tensor.matmul(psZN[:, (2 + i) * T:(3 + i) * T], w_hn,
                                 t2[:, (2 * i + 1) * T:(2 * i + 2) * T],
                                 start=False, stop=True)
            zn = apool.tile([HID, 4 * T], BF16, name="zn_int", tag="act")
            nc.scalar.activation(zn[:, 0:2 * T], psZN[:, 0:2 * T], SIGM,
                                 **kw_b(b_zneg if with_bias else None))
            nc.scalar.activation(zn[:, 2 * T:4 * T], psZN[:, 2 * T:4 * T],
                                 TANH, **kw_b(b_n if with_bias else None))
            # h = cs + zz*(n - cs)
            u = tpool.tile([HID, 2 * T], BF16, name="u_sb", tag="u")
            nc.vector.tensor_sub(u, zn[:, 2 * T:4 * T], cs)
            v = tpool.tile([HID, 2 * T], BF16, name="v_sb", tag="v")
            nc.vector.tensor_mul(v, zn[:, 0:2 * T], u)
            nc.vector.tensor_add(h_lvl[l][:, t0:t0 + 2 * T], v, cs)
            if with_mask:
                mask_mul(h_lvl[l][:, t0:t0 + 2 * T], l, t0, 2 * T)

        # ---------------- tail solo unit (one tile of R parents) --------
        def solo_unit(l):
            Rp = _R(l)
            ch = h_lvl[l + 1]
            if l > SMALL_MAX_LVL:
                xp = xpool.tile([128, Rp], BF16, name="xp_tail", tag="xp")
                nc.sync.dma_start(
                    out=xp, in_=xint_d.ap()[:, int_off[l]:int_off[l] + Rp])
            else:
                xp = xsmall[:, small_off[l]:small_off[l] + Rp]
            cs = tpool.tile([HID, Rp], BF16, name="cs_t", tag="cs")
            nc.vector.tensor_add(cs, ch[:, 0:Rp], ch[:, Rp:2 * Rp])
            # second accumulation region at col 512 -> its own PSUM bank
            hoff = 512
            psr = pspool.tile([HID, 2048], F32, name="psr_t", tag="psr")
            for s, o in ((0, 0), (1, hoff)):
                nc.tensor.matmul(psr[:, o:o + Rp],
                                 wx[32 * s:32 * (s + 1)],
                                 xp[32 * s:32 * (s + 1)],
                                 start=True, stop=False,
                                 tile_position=(32 * s, 0))
            nc.tensor.matmul(psr[:, 0:Rp], w_hr, ch[:, 0:Rp],
                             start=False, stop=True)
            nc.tensor.matmul(psr[:, hoff:hoff + Rp], w_hr, ch[:, Rp:2 * Rp],
                             start=False, stop=True)
            r_sb = apool.tile([HID, 2 * Rp], BF16, name="r_tail", tag="act")
            psr_v = psr.rearrange("p (g f) -> p g f", g=4)[:, 0:2, 0:Rp]
            r_v = r_sb.rearrange("p (g f) -> p g f", g=2)
            nc.scalar.activation(r_v, psr_v, SIGM,
                                 **kw_b(b_r if with_bias else None))
            t2 = tpool.tile([HID, 2 * Rp], BF16, name="t2_t", tag="t2")
            ch2 = ch.rearrange("p (g f) -> p g f", g=2)[:, :, 0:Rp]
            nc.vector.tensor_mul(t2.rearrange("p (g f) -> p g f", g=2),
                                 r_sb.rearrange("p (g f) -> p g f", g=2), ch2)
            psZN = pspool.tile([HID, 2048], F32, name="psZN_t", tag="psz")
            nc.tensor.matmul(psZN[:, 0:Rp], wx[64:96], xp[64:96],
                             start=True, stop=False, tile_position=(64, 0))
            nc.tensor.matmul(psZN[:, hoff:hoff + Rp], wx[96:128], xp[96:128],
                             start=True, stop=False, tile_position=(96, 0))
            nc.tensor.matmul(psZN[:, 0:Rp], w_hz, cs, start=False, stop=True)
            nc.tensor.matmul(psZN[:, hoff:hoff + Rp], w_hn, t2[:, 0:Rp],
                             start=False, stop=False)
            nc.tensor.matmul(psZN[:, hoff:hoff + Rp], w_hn, t2[:, Rp:2 * Rp],
                             start=False, stop=True)
            zn = apool.tile([HID, 2 * Rp], BF16, name="zn_tail", tag="act")
            nc.scalar.activation(zn[:, 0:Rp], psZN[:, 0:Rp], SIGM,
                                 **kw_b(b_zneg if with_bias else None))
            nc.scalar.activation(zn[:, Rp:2 * Rp], psZN[:, hoff:hoff + Rp],
                                 TANH, **kw_b(b_n if with_bias else None))
            u = tpool.tile([HID, Rp], BF16, name="u_t", tag="u")
            v = tpool.tile([HID, Rp], BF16, name="v_t", tag="v")
            nc.vector.tensor_sub(u, zn[:, Rp:2 * Rp], cs)
            nc.vector.tensor_mul(v, zn[:, 0:Rp], u)
            nc.vector.tensor_add(h_lvl[l][:, 0:Rp], v, cs)
            if with_mask:
                mask_mul(h_lvl[l][:, 0:Rp], l, 0, Rp)

        # ---------------- emission ----------------
        for k in range(n_units):
            if k == 2:
                nc.sync.dma_start(out=xsmall, in_=xsmall_d.ap())
                nc.sync.dma_start(out=w_out, in_=w_out_d.ap())
            leaf_unit(k)
        for l in range(DEPTH - 2, 8, -1):
            for j in range(_R(l) // T_TILE // 2):
                pair_unit(l, j)
        for l in range(8, -1, -1):
            solo_unit(l)

        # ---------------- output head ----------------
        h0f = tpool.tile([HID, B_LOCAL], F32, name="h0f", tag="h0f")
        nc.vector.tensor_copy(h0f, h_lvl[0])
        ps_out = pspool.tile([HID, B_LOCAL], F32, name="ps_out", tag="psr")
        nc.tensor.matmul(ps_out, w_out, h0f, start=True, stop=True)
        out_sb = opool.tile([HID, B_LOCAL], F32, name="out_sb", tag="out_sb")
        if with_bias:
            nc.scalar.activation(out_sb, ps_out,
                                 mybir.ActivationFunctionType.Identity,
                                 bias=b_out)
        else:
            nc.scalar.copy(out_sb, ps_out)
        nc.sync.dma_start(out=out_d.ap(), in_=out_sb)

    nc.compile()
    return nc


def host_prep(inputs, with_mask=False, with_bias=False):
    t = np.ascontiguousarray(np.asarray(inputs["targets"], np.float32))
    N = t.shape[0]
    assert N == 2**DEPTH - 1 and t.shape[2] == IN_DIM
    leaf = DEPTH - 1

    xt = np.ascontiguousarray(t.transpose(2, 0, 1)).astype(BF16NP)
    revs = {l: _bitrev(l) for l in range(DEPTH)}

    def plain_t(w):
        return np.ascontiguousarray(np.asarray(w, np.float32).T).astype(BF16NP)

    w_ir = plain_t(inputs["W_ir"])
    w_izn = np.ascontiguousarray(
        -np.asarray(inputs["W_iz"], np.float32).T).astype(BF16NP)
    w_in = plain_t(inputs["W_in"])
    w_out = np.ascontiguousarray(
        np.concatenate([np.asarray(inputs["W_mu"], np.float32),
                        np.asarray(inputs["W_lv"], np.float32)], axis=0).T)

    wcat = np.zeros((128, 5 * HID), BF16NP)
    wcat[:, 0 * HID:1 * HID] = plain_t(inputs["W_hr"])
    wcat[:, 1 * HID:2 * HID] = np.ascontiguousarray(
        -np.asarray(inputs["W_hz"], np.float32).T).astype(BF16NP)
    wcat[:, 2 * HID:3 * HID] = plain_t(inputs["W_hn"])
    for i, wsrc in enumerate((w_ir, w_ir, w_izn, w_in)):
        wcat[32 * i:32 * (i + 1), 3 * HID:4 * HID] = wsrc
    for i, wsrc in enumerate((w_izn, w_izn, w_in, w_in)):
        wcat[32 * i:32 * (i + 1), 4 * HID:5 * HID] = wsrc

    shared = dict(wcat=wcat, w_out=w_out)
    if with_bias:
        b = {k: np.asarray(inputs[k], np.float32) for k in
             ("b_ir", "b_hr", "b_iz", "b_hz", "b_in", "b_hn", "b_mu", "b_lv")}
        bias = np.zeros((HID, 6), np.float32)
        bias[:, 0] = b["b_ir"] + b["b_hr"]
        bias[:, 1] = -(b["b_iz"] + b["b_hz"])
        bias[:, 2] = b["b_in"] + b["b_hn"]
        bias[:128, 3] = np.concatenate([b["b_mu"], b["b_lv"]])
        bias[:, 4] = -b["b_iz"]
        bias[:, 5] = b["b_in"]
        shared["biases"] = bias

    int_lvls = list(range(DEPTH - 2, SMALL_MAX_LVL, -1))
    n_units = _R(leaf) // (2 * T_TILE)

    in_maps = []
    for c in range(N_CORES):
        b0 = c * B_LOCAL
        xc = xt[:, :, b0:b0 + B_LOCAL]
        xl = {}
        for l in range(DEPTH):
            start = 2**l - 1
            blk = xc[:, start + revs[l], :]
            xl[l] = np.ascontiguousarray(blk.reshape(IN_DIM, _R(l)))

        xint = np.concatenate([np.tile(xl[l], (4, 1)) for l in int_lvls],
                              axis=1)
        half = _R(leaf) // 2
        lblocks = []
        for k in range(n_units):
            xA = xl[leaf][:, k * T_TILE:(k + 1) * T_TILE]
            xB = xl[leaf][:, half + k * T_TILE:half + (k + 1) * T_TILE]
            lblocks.append(np.concatenate([xA, xB, xA, xB], axis=0))
        xleaf = np.concatenate(lblocks, axis=1)
        xsmall = np.concatenate([np.tile(xl[l], (4, 1))
                                 for l in range(SMALL_MAX_LVL, -1, -1)],
                                axis=1)
        m = dict(shared)
        m["xint"] = np.ascontiguousarray(xint)
        m["xleaf"] = np.ascontiguousarray(xleaf)
        m["xsmall"] = np.ascontiguousarray(xsmall)
        if with_mask:
            mk = np.asarray(inputs["mask"], np.float32)[:, b0:b0 + B_LOCAL]
            mblocks = []
            for l in range(DEPTH):
                start = 2**l - 1
                mblocks.append(mk[start + revs[l], :].reshape(1, _R(l)))
            mcat = np.concatenate(mblocks, axis=1)
            m["mask_bc"] = np.ascontiguousarray(
                np.broadcast_to(mcat, (HID, mcat.shape[1]))).astype(BF16NP)
        in_maps.append(m)
    return in_maps


_PROGRAM_CACHE = {}


def _get_program(with_mask, with_bias):
    key = (with_mask, with_bias)
    if key not in _PROGRAM_CACHE:
        _PROGRAM_CACHE[key] = build_program(with_mask=with_mask,
                                            with_bias=with_bias)
    return _PROGRAM_CACHE[key]


def run_on_device(inputs, trace=False, **trace_kw):
    with_mask = not np.all(np.asarray(inputs["mask"]) == 1.0)
    with_bias = any(
        np.any(np.asarray(inputs[k]) != 0.0)
        for k in ("b_ir", "b_hr", "b_iz", "b_hz", "b_in", "b_hn",
                  "b_mu", "b_lv"))
    nc = _get_program(with_mask, with_bias)
    in_maps = host_prep(inputs, with_mask=with_mask, with_bias=with_bias)
    res = bass_utils.run_bass_kernel_spmd(
        nc, in_maps, core_ids=list(range(N_CORES)), trace=trace, **trace_kw)
    mu = np.zeros((BATCH, OUT_DIM), np.float32)
    lv = np.zeros((BATCH, OUT_DIM), np.float32)
    for c in range(N_CORES):
        o = res.results[c]["out"]
        mu[c * B_LOCAL:(c + 1) * B_LOCAL] = o[:OUT_DIM].T
        lv[c * B_LOCAL:(c + 1) * B_LOCAL] = o[OUT_DIM:].T
    return (mu, lv), res


def kernel(**inputs):
    (mu, lv), _ = run_on_device(inputs)
    return mu, lv


# revision 10
# speedup vs baseline: 1.4139x; 1.4139x over previous
"""Trainium2 Bass kernel for nn_Encoder_55490977464569 (binary-tree GRU encoder).

Strategy (v5)
-------------
Data-parallel over batch: B=16 -> 2 batch columns per NeuronCore, zero
collectives. Each core runs its whole tree (32767 nodes) leaves->root with all
hidden states resident in SBUF (bf16); only `targets` is streamed in.

Layout: feature-major [128 features (partitions), node*batch columns], each
level's nodes stored in BIT-REVERSED in-level order. With bit-reversal at
every level, the children of parent tile [t0, t0+T) are planeL =
child[:, t0:t0+T] and planeR = child[:, R_parent+t0 : +T] (both contiguous),
and the parent's h is written back contiguously.

The ScalarE ACTIVATE stream is the bottleneck: every PSUM column needs
exactly one ACT (PSUM->SBUF drain + nonlinearity fused), at 1 col/cycle
@1.2GHz + ~180cy/instr overhead. v5 minimizes instruction count with wide
merged ACTs on 4-bank PSUM tiles:
  - leaves first (not interleaved), one [128,2048] PSUM tile [zA|zB|nA|nB]
    per unit (alternating between the two 4-bank tiles for 2-deep
    pipelining): ONE sigmoid ACT [1024] + ONE tanh ACT [1024], then
    h = zz*n as a single two-region TT.
  - interior pair (2 tiles x T=512 parents): psr [128,2048] holds
    [rlA|rrA|rlB|rrB] -> ONE sigmoid ACT [2048]; psZN [128,2048] holds
    [zA|zB|nA|nB] -> ONE sigmoid [1024] + ONE tanh [1024].
  - W_hz/w_iz are negated on the host so zz = sigmoid(+pre) at scale=1.
Blend h = cs + zz*(n - cs) runs as 3 plain [1024]-wide TTs (DVE 2x mode).
"""

import sys

if "/opt/trn_rl_repo" not in sys.path:
    sys.path.insert(0, "/opt/trn_rl_repo")
if "/opt/trn_rl_repo/concourse" not in sys.path:
    sys.path.insert(0, "/opt/trn_rl_repo/concourse")

import numpy as np
import ml_dtypes

from concourse import bass, mybir, tile, bacc
from concourse import bass_utils

BF16NP = ml_dtypes.bfloat16
F32 = mybir.dt.float32
BF16 = mybir.dt.bfloat16

N_CORES = 8
DEPTH = 15
HID = 128
IN_DIM = 32
OUT_DIM = 64
BATCH = 16
B_LOCAL = BATCH // N_CORES

T_TILE = 512
SMALL_MAX_LVL = 7

ADD = mybir.AluOpType.add
SUB = mybir.AluOpType.subtract
MULT = mybir.AluOpType.mult
TANH = mybir.ActivationFunctionType.Tanh
SIGM = mybir.ActivationFunctionType.Sigmoid


def _R(l):
    return 2**l * B_LOCAL


def _bitrev(n_bits):
    n = 1 << n_bits
    p = np.zeros(n, dtype=np.int64)
    for i in range(n):
        r = 0
        x = i
        for _ in range(n_bits):
            r = (r << 1) | (x & 1)
            x >>= 1
        p[i] = r
    return p


def build_program(with_mask=False, with_bias=False):
    nc = bacc.Bacc("TRN2", target_bir_lowering=False, debug=False,
                   num_devices=1)
    leaf = DEPTH - 1

    int_lvls = list(range(DEPTH - 2, SMALL_MAX_LVL, -1))
    int_off = {}
    off = 0
    for l in int_lvls:
        int_off[l] = off
        off += _R(l)
    xint_d = nc.dram_tensor("xint", [128, off], BF16, kind="ExternalInput")
    n_units = _R(leaf) // (2 * T_TILE)
    xleaf_d = nc.dram_tensor("xleaf", [128, n_units * T_TILE], BF16,
                             kind="ExternalInput")
    small_cols = sum(_R(l) for l in range(SMALL_MAX_LVL + 1))
    xsmall_d = nc.dram_tensor("xsmall", [128, small_cols], BF16,
                              kind="ExternalInput")
    wcat_d = nc.dram_tensor("wcat", [128, 5 * HID], BF16, kind="ExternalInput")
    w_out_d = nc.dram_tensor("w_out", [HID, 2 * OUT_DIM], F32,
                             kind="ExternalInput")
    out_d = nc.dram_tensor("out", [HID, B_LOCAL], F32, kind="ExternalOutput")
    if with_bias:
        bias_d = nc.dram_tensor("biases", [HID, 6], F32, kind="ExternalInput")
    if with_mask:
        total_z = sum(_R(l) for l in range(DEPTH))
        mask_d = nc.dram_tensor("mask_bc", [HID, total_z], BF16,
                                kind="ExternalInput")
        mask_off = {}
        moff = 0
        for l in range(DEPTH):
            mask_off[l] = moff
            moff += _R(l)

    from contextlib import ExitStack
    with tile.TileContext(nc) as tc, ExitStack() as stack:
        consts = stack.enter_context(tc.tile_pool(name="consts", bufs=1))
        hpool = stack.enter_context(tc.tile_pool(name="hpool", bufs=1))
        xpool = stack.enter_context(tc.tile_pool(name="xpool", bufs=6))
        apool = stack.enter_context(tc.tile_pool(name="apool", bufs=4))
        tpool = stack.enter_context(tc.tile_pool(name="tpool", bufs=4))
        pspool = stack.enter_context(tc.tile_pool(name="pspool", bufs=1,
                                                  space="PSUM"))
        opool = stack.enter_context(tc.tile_pool(name="opool", bufs=1))

        wcat_sb = consts.tile([128, 5 * HID], BF16, name="wcat_sb",
                              tag="wcat_sb")
        nc.sync.dma_start(out=wcat_sb, in_=wcat_d.ap())
        w_hr = wcat_sb[:, 0 * HID:1 * HID]
        w_hz = wcat_sb[:, 1 * HID:2 * HID]   # negated on host
        w_hn = wcat_sb[:, 2 * HID:3 * HID]
        wx = wcat_sb[:, 3 * HID:4 * HID]   # [w_ir; w_ir; -w_iz; w_in]
        wl = wcat_sb[:, 4 * HID:5 * HID]   # [-w_iz; -w_iz; w_in; w_in]
        w_out = consts.tile([HID, 2 * OUT_DIM], F32, name="w_out_sb",
                            tag="w_out_sb")
        xsmall = consts.tile([128, small_cols], BF16, name="xsmall",
                             tag="xsmall")
        # (their DMAs are emitted after the first leaf units, off the
        # startup critical path)
        small_off = {}
        soff = 0
        for l in range(SMALL_MAX_LVL, -1, -1):
            small_off[l] = soff
            soff += _R(l)
        if with_bias:
            bias_sb = consts.tile([HID, 6], F32, name="bias_sb", tag="bias_sb")
            nc.sync.dma_start(out=bias_sb, in_=bias_d.ap())
            b_r = bias_sb[:, 0:1]      # b_ir + b_hr
            b_zneg = bias_sb[:, 1:2]   # -(b_iz + b_hz)
            b_n = bias_sb[:, 2:3]      # b_in + b_hn
            b_out = bias_sb[:, 3:4]
            b_lz = bias_sb[:, 4:5]     # -b_iz  (leaf z)
            b_ln = bias_sb[:, 5:6]     # b_in   (leaf n)

        h_lvl = [hpool.tile([HID, _R(l)], BF16, name=f"h_{l}", tag=f"h_{l}")
                 for l in range(DEPTH)]

        def mask_mul(view, lvl, col0, width):
            m_sb = tpool.tile([HID, width], BF16, name="m_sb", tag="m_sb")
            nc.sync.dma_start(
                out=m_sb,
                in_=mask_d.ap()[:, mask_off[lvl] + col0:
                                mask_off[lvl] + col0 + width])
            nc.vector.tensor_mul(view, view, m_sb)

        def kw_b(b):
            return dict(bias=b) if with_bias else {}

        # ---------------- leaf units ----------------
        # unit k covers tiles A = cols [kT,(k+1)T) and B = half + same.
        # Even units use the 4-bank psr tile [zA|zB|nA|nB]; odd units use
        # psZ [zA|zB] + psN [nA|nB] -- 2-deep pipelining within 8 banks.
        def leaf_unit(k):
            T = T_TILE
            xp = xpool.tile([128, T], BF16, name="xp_leaf", tag="xp")
            nc.sync.dma_start(out=xp, in_=xleaf_d.ap()[:, k * T:(k + 1) * T])
            if k % 2 == 0:
                ps = pspool.tile([HID, 2048], F32, name="psL", tag="psr")
                dsts = [ps[:, s * T:(s + 1) * T] for s in range(4)]
                z_src, n_src = ps[:, 0:2 * T], ps[:, 2 * T:4 * T]
            else:
                psZ = pspool.tile([HID, 1024], F32, name="psLZ", tag="psZ")
                psN = pspool.tile([HID, 1024], F32, name="psLN", tag="psN")
                dsts = [psZ[:, 0:T], psZ[:, T:2 * T],
                        psN[:, 0:T], psN[:, T:2 * T]]
                z_src, n_src = psZ, psN
            # strips of xp are [xA; xB; xA; xB], wl = [-w_iz;-w_iz;w_in;w_in]
            for s in range(4):
                nc.tensor.matmul(dsts[s], wl[32 * s:32 * (s + 1)],
                                 xp[32 * s:32 * (s + 1)],
                                 start=True, stop=True,
                                 tile_position=(32 * s, 0))
            zn = apool.tile([HID, 4 * T], BF16, name="zn_leaf", tag="act")
            nc.scalar.activation(zn[:, 0:2 * T], z_src, SIGM,
                                 **kw_b(b_lz if with_bias else None))
            nc.scalar.activation(zn[:, 2 * T:4 * T], n_src, TANH,
                                 **kw_b(b_ln if with_bias else None))
            hv = h_lvl[leaf]
            half = _R(leaf) // 2
            hv2 = hv.rearrange("p (g f) -> p g f", g=2)[:, :, k * T:(k + 1) * T]
            zzv = zn[:, 0:2 * T].rearrange("p (g f) -> p g f", g=2)
            nv = zn[:, 2 * T:4 * T].rearrange("p (g f) -> p g f", g=2)
            nc.vector.tensor_mul(hv2, zzv, nv)
            if with_mask:
                mask_mul(hv[:, k * T:(k + 1) * T], leaf, k * T, T)
                mask_mul(hv[:, half + k * T:half + (k + 1) * T], leaf,
                         half + k * T, T)

        # ------- interior pair (2 tiles of T parents), skewed emission ----
        # front: cs, x DMAs, r matmuls, r-ACT, t2.  back: z/n matmuls,
        # zz/n ACTs, blend.  Emitting front(i+1) before back(i) keeps the
        # ACT stream dense: r(i+1) runs while back-matmuls of i finish.
        def pair_front(l, j):
            T = T_TILE
            t0 = 2 * j * T
            ch = h_lvl[l + 1]
            R = _R(l)
            if l > SMALL_MAX_LVL:
                xpA = xpool.tile([128, T], BF16, name="xp_intA", tag="xp")
                nc.sync.dma_start(
                    out=xpA, in_=xint_d.ap()[:, int_off[l] + t0:
                                             int_off[l] + t0 + T])
                xpB = xpool.tile([128, T], BF16, name="xp_intB", tag="xp")
                nc.sync.dma_start(
                    out=xpB, in_=xint_d.ap()[:, int_off[l] + t0 + T:
                                             int_off[l] + t0 + 2 * T])
            else:
                xpA = xsmall[:, small_off[l] + t0:small_off[l] + t0 + T]
                xpB = xsmall[:, small_off[l] + t0 + T:small_off[l] + t0 + 2 * T]
            cs = tpool.tile([HID, 2 * T], BF16, name="cs", tag="cs")
            nc.vector.tensor_add(cs, ch[:, t0:t0 + 2 * T],
                                 ch[:, R + t0:R + t0 + 2 * T])
            # psr = [rlA | rrA | rlB | rrB]
            psr = pspool.tile([HID, 2048], F32, name="psr", tag="psr")
            for i, xp in enumerate((xpA, xpB)):
                o = 2 * i * T
                for s in range(2):
                    nc.tensor.matmul(psr[:, o + s * T:o + (s + 1) * T],
                                     wx[32 * s:32 * (s + 1)],
                                     xp[32 * s:32 * (s + 1)],
                                     start=True, stop=False,
                                     tile_position=(32 * s, 0))
                nc.tensor.matmul(psr[:, o:o + T], w_hr,
                                 ch[:, t0 + i * T:t0 + (i + 1) * T],
                                 start=False, stop=True)
                nc.tensor.matmul(psr[:, o + T:o + 2 * T], w_hr,
                                 ch[:, R + t0 + i * T:R + t0 + (i + 1) * T],
                                 start=False, stop=True)
            r_sb = apool.tile([HID, 4 * T], BF16, name="r_sb", tag="act")
            nc.scalar.activation(r_sb, psr, SIGM,
                                 **kw_b(b_r if with_bias else None))
            # t2 = r * h_child (two-region child APs, one TT per tile)
            t2 = tpool.tile([HID, 4 * T], BF16, name="t2", tag="t2")
            ch2 = ch.rearrange("p (g f) -> p g f", g=2)
            for i in range(2):
                sl = slice(2 * i * T, (2 * i + 2) * T)
                nc.vector.tensor_mul(
                    t2[:, sl].rearrange("p (g f) -> p g f", g=2),
                    r_sb[:, sl].rearrange("p (g f) -> p g f", g=2),
                    ch2[:, :, t0 + i * T:t0 + (i + 1) * T])
            return dict(l=l, t0=t0, cs=cs, t2=t2, xpA=xpA, xpB=xpB)

        def pair_back(st):
            T = T_TILE
            l, t0, cs, t2 = st["l"], st["t0"], st["cs"], st["t2"]
            psZ = pspool.tile([HID, 1024], F32, name="psZ", tag="psZ")
            psN = pspool.tile([HID, 1024], F32, name="psN", tag="psN")
            for i, xp in enumerate((st["xpA"], st["xpB"])):
                nc.tensor.matmul(psZ[:, i * T:(i + 1) * T],
                                 wx[64:96], xp[64:96],
                                 start=True, stop=False,
                                 tile_position=(64, 0))
                nc.tensor.matmul(psN[:, i * T:(i + 1) * T],
                                 wx[96:128], xp[96:128],
                                 start=True, stop=False,
                                 tile_position=(96, 0))
            for i in range(2):
                nc.tensor.matmul(psZ[:, i * T:(i + 1) * T], w_hz,
                                 cs[:, i * T:(i + 1) * T],
                                 start=False, stop=True)
                nc.tensor.matmul(psN[:, i * T:(i + 1) * T], w_hn,
                                 t2[:, 2 * i * T:(2 * i + 1) * T],
                                 start=False, stop=False)
                nc.tensor.matmul(psN[:, i * T:(i + 1) * T], w_hn,
                                 t2[:, (2 * i + 1) * T:(2 * i + 2) * T],
                                 start=False, stop=True)
            zn = apool.tile([HID, 4 * T], BF16, name="zn_int", tag="act")
            nc.scalar.activation(zn[:, 0:2 * T], psZ, SIGM,
                                 **kw_b(b_zneg if with_bias else None))
            nc.scalar.activation(zn[:, 2 * T:4 * T], psN, TANH,
                                 **kw_b(b_n if with_bias else None))
            # h = cs + zz*(n - cs)
            u = tpool.tile([HID, 2 * T], BF16, name="u_sb", tag="u")
            nc.vector.tensor_sub(u, zn[:, 2 * T:4 * T], cs)
            v = tpool.tile([HID, 2 * T], BF16, name="v_sb", tag="v")
            nc.vector.tensor_mul(v, zn[:, 0:2 * T], u)
            nc.vector.tensor_add(h_lvl[l][:, t0:t0 + 2 * T], v, cs)
            if with_mask:
                mask_mul(h_lvl[l][:, t0:t0 + 2 * T], l, t0, 2 * T)

        # ---------------- tail solo unit (one tile of R parents) --------
        def solo_unit(l):
            Rp = _R(l)
            ch = h_lvl[l + 1]
            if l > SMALL_MAX_LVL:
                xp = xpool.tile([128, Rp], BF16, name="xp_tail", tag="xp")
                nc.sync.dma_start(
                    out=xp, in_=xint_d.ap()[:, int_off[l]:int_off[l] + Rp])
            else:
                xp = xsmall[:, small_off[l]:small_off[l] + Rp]
            cs = tpool.tile([HID, Rp], BF16, name="cs_t", tag="cs")
            nc.vector.tensor_add(cs, ch[:, 0:Rp], ch[:, Rp:2 * Rp])
            # second accumulation region at col 512 -> its own PSUM bank
            hoff = 512
            psr = pspool.tile([HID, 2048], F32, name="psr_t", tag="psr")
            for s, o in ((0, 0), (1, hoff)):
                nc.tensor.matmul(psr[:, o:o + Rp],
                                 wx[32 * s:32 * (s + 1)],
                                 xp[32 * s:32 * (s + 1)],
                                 start=True, stop=False,
                                 tile_position=(32 * s, 0))
            nc.tensor.matmul(psr[:, 0:Rp], w_hr, ch[:, 0:Rp],
                             start=False, stop=True)
            nc.tensor.matmul(psr[:, hoff:hoff + Rp], w_hr, ch[:, Rp:2 * Rp],
                             start=False, stop=True)
            r_sb = apool.tile([HID, 2 * Rp], BF16, name="r_tail", tag="act")
            psr_v = psr.rearrange("p (g f) -> p g f", g=4)[:, 0:2, 0:Rp]
            r_v = r_sb.rearrange("p (g f) -> p g f", g=2)
            nc.scalar.activation(r_v, psr_v, SIGM,
                                 **kw_b(b_r if with_bias else None))
            t2 = tpool.tile([HID, 2 * Rp], BF16, name="t2_t", tag="t2")
            ch2 = ch.rearrange("p (g f) -> p g f", g=2)[:, :, 0:Rp]
            nc.vector.tensor_mul(t2.rearrange("p (g f) -> p g f", g=2),
                                 r_sb.rearrange("p (g f) -> p g f", g=2), ch2)
            psZ = pspool.tile([HID, 1024], F32, name="psZ_t", tag="psZ")
            psN = pspool.tile([HID, 1024], F32, name="psN_t", tag="psN")
            nc.tensor.matmul(psZ[:, 0:Rp], wx[64:96], xp[64:96],
                             start=True, stop=False, tile_position=(64, 0))
            nc.tensor.matmul(psN[:, 0:Rp], wx[96:128], xp[96:128],
                             start=True, stop=False, tile_position=(96, 0))
            nc.tensor.matmul(psZ[:, 0:Rp], w_hz, cs, start=False, stop=True)
            nc.tensor.matmul(psN[:, 0:Rp], w_hn, t2[:, 0:Rp],
                             start=False, stop=False)
            nc.tensor.matmul(psN[:, 0:Rp], w_hn, t2[:, Rp:2 * Rp],
                             start=False, stop=True)
            zn = apool.tile([HID, 2 * Rp], BF16, name="zn_tail", tag="act")
            nc.scalar.activation(zn[:, 0:Rp], psZ[:, 0:Rp], SIGM,
                                 **kw_b(b_zneg if with_bias else None))
            nc.scalar.activation(zn[:, Rp:2 * Rp], psN[:, 0:Rp],
                                 TANH, **kw_b(b_n if with_bias else None))
            u = tpool.tile([HID, Rp], BF16, name="u_t", tag="u")
            v = tpool.tile([HID, Rp], BF16, name="v_t", tag="v")
            nc.vector.tensor_sub(u, zn[:, Rp:2 * Rp], cs)
            nc.vector.tensor_mul(v, zn[:, 0:Rp], u)
            nc.vector.tensor_add(h_lvl[l][:, 0:Rp], v, cs)
            if with_mask:
                mask_mul(h_lvl[l][:, 0:Rp], l, 0, Rp)

        # ---------------- emission ----------------
        for k in range(n_units):
            if k == 2:
                nc.sync.dma_start(out=xsmall, in_=xsmall_d.ap())
                nc.sync.dma_start(out=w_out, in_=w_out_d.ap())
            leaf_unit(k)
        prev = None
        for l in range(DEPTH - 2, 8, -1):
            if l == 9 and prev is not None:
                # skew flush: front(9,0) reads the last level-10 pair's h
                pair_back(prev)
                prev = None
            for j in range(_R(l) // T_TILE // 2):
                st = pair_front(l, j)
                if prev is not None:
                    pair_back(prev)
                prev = st
        pair_back(prev)
        for l in range(8, -1, -1):
            solo_unit(l)

        # ---------------- output head ----------------
        h0f = tpool.tile([HID, B_LOCAL], F32, name="h0f", tag="h0f")
        nc.vector.tensor_copy(h0f, h_lvl[0])
        ps_out = pspool.tile([HID, B_LOCAL], F32, name="ps_out", tag="psr")
        nc.tensor.matmul(ps_out, w_out, h0f, start=True, stop=True)
        out_sb = opool.tile([HID, B_LOCAL], F32, name="out_sb", tag="out_sb")
        if with_bias:
            nc.scalar.activation(out_sb, ps_out,
                                 mybir.ActivationFunctionType.Identity,
                                 bias=b_out)
        else:
            nc.scalar.copy(out_sb, ps_out)
        nc.sync.dma_start(out=out_d.ap(), in_=out_sb)

    nc.compile()
    return nc


def host_prep(inputs, with_mask=False, with_bias=False):
    t = np.ascontiguousarray(np.asarray(inputs["targets"], np.float32))
    N = t.shape[0]
    assert N == 2**DEPTH - 1 and t.shape[2] == IN_DIM
    leaf = DEPTH - 1

    xt = np.ascontiguousarray(t.transpose(2, 0, 1)).astype(BF16NP)
    revs = {l: _bitrev(l) for l in range(DEPTH)}

    def plain_t(w):
        return np.ascontiguousarray(np.asarray(w, np.float32).T).astype(BF16NP)

    w_ir = plain_t(inputs["W_ir"])
    w_izn = np.ascontiguousarray(
        -np.asarray(inputs["W_iz"], np.float32).T).astype(BF16NP)
    w_in = plain_t(inputs["W_in"])
    w_out = np.ascontiguousarray(
        np.concatenate([np.asarray(inputs["W_mu"], np.float32),
                        np.asarray(inputs["W_lv"], np.float32)], axis=0).T)

    wcat = np.zeros((128, 5 * HID), BF16NP)
    wcat[:, 0 * HID:1 * HID] = plain_t(inputs["W_hr"])
    wcat[:, 1 * HID:2 * HID] = np.ascontiguousarray(
        -np.asarray(inputs["W_hz"], np.float32).T).astype(BF16NP)
    wcat[:, 2 * HID:3 * HID] = plain_t(inputs["W_hn"])
    for i, wsrc in enumerate((w_ir, w_ir, w_izn, w_in)):
        wcat[32 * i:32 * (i + 1), 3 * HID:4 * HID] = wsrc
    for i, wsrc in enumerate((w_izn, w_izn, w_in, w_in)):
        wcat[32 * i:32 * (i + 1), 4 * HID:5 * HID] = wsrc

    shared = dict(wcat=wcat, w_out=w_out)
    if with_bias:
        b = {k: np.asarray(inputs[k], np.float32) for k in
             ("b_ir", "b_hr", "b_iz", "b_hz", "b_in", "b_hn", "b_mu", "b_lv")}
        bias = np.zeros((HID, 6), np.float32)
        bias[:, 0] = b["b_ir"] + b["b_hr"]
        bias[:, 1] = -(b["b_iz"] + b["b_hz"])
        bias[:, 2] = b["b_in"] + b["b_hn"]
        bias[:128, 3] = np.concatenate([b["b_mu"], b["b_lv"]])
        bias[:, 4] = -b["b_iz"]
        bias[:, 5] = b["b_in"]
        shared["biases"] = bias

    int_lvls = list(range(DEPTH - 2, SMALL_MAX_LVL, -1))
    n_units = _R(leaf) // (2 * T_TILE)

    in_maps = []
    for c in range(N_CORES):
        b0 = c * B_LOCAL
        xc = xt[:, :, b0:b0 + B_LOCAL]
        xl = {}
        for l in range(DEPTH):
            start = 2**l - 1
            blk = xc[:, start + revs[l], :]
            xl[l] = np.ascontiguousarray(blk.reshape(IN_DIM, _R(l)))

        xint = np.concatenate([np.tile(xl[l], (4, 1)) for l in int_lvls],
                              axis=1)
        half = _R(leaf) // 2
        lblocks = []
        for k in range(n_units):
            xA = xl[leaf][:, k * T_TILE:(k + 1) * T_TILE]
            xB = xl[leaf][:, half + k * T_TILE:half + (k + 1) * T_TILE]
            lblocks.append(np.concatenate([xA, xB, xA, xB], axis=0))
        xleaf = np.concatenate(lblocks, axis=1)
        xsmall = np.concatenate([np.tile(xl[l], (4, 1))
                                 for l in range(SMALL_MAX_LVL, -1, -1)],
                                axis=1)
        m = dict(shared)
        m["xint"] = np.ascontiguousarray(xint)
        m["xleaf"] = np.ascontiguousarray(xleaf)
        m["xsmall"] = np.ascontiguousarray(xsmall)
        if with_mask:
            mk = np.asarray(inputs["mask"], np.float32)[:, b0:b0 + B_LOCAL]
            mblocks = []
            for l in range(DEPTH):
                start = 2**l - 1
                mblocks.append(mk[start + revs[l], :].reshape(1, _R(l)))
            mcat = np.concatenate(mblocks, axis=1)
            m["mask_bc"] = np.ascontiguousarray(
                np.broadcast_to(mcat, (HID, mcat.shape[1]))).astype(BF16NP)
        in_maps.append(m)
    return in_maps


_PROGRAM_CACHE = {}


def _get_program(with_mask, with_bias):
    key = (with_mask, with_bias)
    if key not in _PROGRAM_CACHE:
        _PROGRAM_CACHE[key] = build_program(with_mask=with_mask,
                                            with_bias=with_bias)
    return _PROGRAM_CACHE[key]


def run_on_device(inputs, trace=False, **trace_kw):
    with_mask = not np.all(np.asarray(inputs["mask"]) == 1.0)
    with_bias = any(
        np.any(np.asarray(inputs[k]) != 0.0)
        for k in ("b_ir", "b_hr", "b_iz", "b_hz", "b_in", "b_hn",
                  "b_mu", "b_lv"))
    nc = _get_program(with_mask, with_bias)
    in_maps = host_prep(inputs, with_mask=with_mask, with_bias=with_bias)
    res = bass_utils.run_bass_kernel_spmd(
        nc, in_maps, core_ids=list(range(N_CORES)), trace=trace, **trace_kw)
    mu = np.zeros((BATCH, OUT_DIM), np.float32)
    lv = np.zeros((BATCH, OUT_DIM), np.float32)
    for c in range(N_CORES):
        o = res.results[c]["out"]
        mu[c * B_LOCAL:(c + 1) * B_LOCAL] = o[:OUT_DIM].T
        lv[c * B_LOCAL:(c + 1) * B_LOCAL] = o[OUT_DIM:].T
    return (mu, lv), res


def kernel(**inputs):
    (mu, lv), _ = run_on_device(inputs)
    return mu, lv
